# revision 24
# baseline (speedup 1.0000x reference)
"""Trainium2 Bass kernel for nn_MemoryAwareAKTAttention.

Math (per batch b):
    integrated = concat([x, mem], -1) @ Wm.T + bm          [S, E]
    q, k, v    = heads(integrated @ W{q,k,v}.T)            [H, S, D]
    scores     = q @ k.T / sqrt(D)                         [H, S, S]
    decay      = exp(-softplus(gamma_h) * |i-j|)           [H, S, S]
    total      = clip(exp(scores * decay), 1e-5, 1e5)
    attn       = total / (sum_j total + 1e-9)
    out        = (attn @ v) @ Wo.T + bo                    [S, E]

Structure exploited: decay underflows to exactly 0.0 a few dozen columns off
the diagonal, so total == 1.0f bit-exactly outside a narrow band (exp(x)
rounds to 1.0f for |x| < 2^-25).  Per 128-row query block only a ~168-wide
strip is computed (scores matmul + exp); a persistent [128, S] total tile
holds 1.0 elsewhere.  attn is a single row-scale of that tile.  For
out = attn @ v, the off-band part folds into a per-head column-sum of v
(rank-1 colsum x r^T), and the row sums fall out of the banded matmul via a
ones-column appended to v: sum_j (total-1)*r = 1 - S*r.  The banded matmul
contracts 256-wide windows, half-chunk shifted, against a row-shifted copy
of v built once with two PE shift-matmuls.

Matmuls run in float32r (fp32 is 4 cycles/row, f32r is 2); the exp and
normalization chain stays fp32.  attn comes out ~6e-4, out ~4e-4 max rel err.

Sharding: data-parallel over batch, one batch element per NeuronCore (B=8).
"""

import os
import numpy as np

import concourse.bass as bass
import concourse.mybir as mybir
import concourse.tile as tile
from concourse import bacc
from concourse.bass_utils import run_bass_kernel_spmd
from concourse.masks import make_identity

F32 = mybir.dt.float32
F32R = mybir.dt.float32r
BF16 = mybir.dt.bfloat16
BAND_DT = F32R     # dtype of the (total-1)*r band / v / colsum path

AF = mybir.ActivationFunctionType
ALU = mybir.AluOpType

B, S, E = 8, 1024, 512
H = 8
D = E // H          # 64
TWO_E = 2 * E
NCORES = 8
SQ = S // 128       # query blocks per core

LAST_PERF = {}      # filled by kernel(): exec_time_ns etc.
_CACHE = {}         # (W, sw) -> compiled Bacc module


def _wstart(qb: int, W: int) -> int:
    """Window start for query block qb, clipped to [0, S-W].

    W=256 windows are half-chunk shifted: [qb*128-64, qb*128+192).
    Odd-chunk windows are chunk-aligned and centered.
    """
    n = W // 128
    if n == 2:
        return min(max(qb * 128 - 64, 0), S - W)
    return min(max((qb - (n - 1) // 2) * 128, 0), S - W)


def _strip(qb: int, W: int, sw: int) -> int:
    """Start of the compute strip (columns where decay can exceed 2^-31)."""
    dcut = (sw - 128) // 2
    lo = min(max(qb * 128 - dcut, 0), S - sw)
    ws = _wstart(qb, W)
    return min(max(lo, ws), ws + W - sw)


def _pick_window(g_min: float) -> int:
    """Smallest window such that off-window total == 1.0f exactly.

    need exp(-g*margin) * SMAX < 2^-25 with SMAX = 64; ln(64 * 2^26) ~ 22.18.
    """
    for n, margin in ((2, 65), (3, 129), (5, 257), (7, 385)):
        if g_min * margin >= 22.18:
            return n * 128
    return S  # dense fallback: every column computed exactly


def _build(W: int, sw: int) -> bacc.Bacc:
    nc = bacc.Bacc(None)
    NW = W // 128
    shifted = (NW == 2)
    half = (NW - 1) // 2
    NSLOT = 2 * half + 1

    combT_d = nc.dram_tensor("combT", [TWO_E, S], F32R, kind="ExternalInput")
    WmT_d = nc.dram_tensor("WmT", [TWO_E, E], F32R, kind="ExternalInput")
    WqT_d = nc.dram_tensor("WqT", [E, E], F32R, kind="ExternalInput")
    WkT_d = nc.dram_tensor("WkT", [E, E], F32R, kind="ExternalInput")
    WvT_d = nc.dram_tensor("WvT", [E, E], F32R, kind="ExternalInput")
    WoT_d = nc.dram_tensor("WoT", [E, E], F32R, kind="ExternalInput")
    bm_d = nc.dram_tensor("bm", [E], F32, kind="ExternalInput")
    bo_d = nc.dram_tensor("bo", [E], F32R, kind="ExternalInput")
    gam_d = nc.dram_tensor("gam", [H], F32, kind="ExternalInput")
    out_d = nc.dram_tensor("out", [S, E], F32, kind="ExternalOutput")
    attn_d = nc.dram_tensor("attn", [H, S, S], F32, kind="ExternalOutput")

    with tile.TileContext(nc) as tc:
        with (
            tc.tile_pool(name="const", bufs=1) as const,
            tc.tile_pool(name="persist", bufs=1) as persist,
        ):
            # ---- constants ------------------------------------------------
            ident_f = const.tile([128, 128], F32)
            make_identity(nc, ident_f[:])
            ident = const.tile([128, 128], F32R)
            nc.vector.tensor_copy(ident[:], ident_f[:])
            ones_fl = const.tile([128, S], F32)
            nc.vector.memset(ones_fl[:], 1.0)
            ones_row = const.tile([1, 128], F32R)
            nc.vector.tensor_copy(ones_row[:], ones_fl[0:1, 0:128])
            zeros_f = const.tile([128, 128], F32)
            nc.vector.memset(zeros_f[:], 0.0)
            zeros_a = const.tile([1, 65], BF16)
            nc.vector.tensor_copy(zeros_a[:], zeros_f[0:1, 0:65])
            zrow = const.tile([1, 512], BF16)
            nc.vector.tensor_copy(zrow[:], ones_fl[0:1, 0:512])
            ones_col = const.tile([128, 1], BAND_DT)
            nc.vector.tensor_copy(ones_col[:], ones_fl[:, 0:1])
            # shift matrices: shd[p, m] = 1{p == m+64}, shu[p, m] = 1{p == m-64}
            shd_f = const.tile([128, 128], F32)
            nc.gpsimd.memset(shd_f[:], 1.0)
            nc.gpsimd.affine_select(
                out=shd_f[:], in_=shd_f[:], compare_op=ALU.is_equal,
                fill=0.0, base=-64, pattern=[[-1, 128]], channel_multiplier=1,
            )
            shu_f = const.tile([128, 128], F32)
            nc.gpsimd.memset(shu_f[:], 1.0)
            nc.gpsimd.affine_select(
                out=shu_f[:], in_=shu_f[:], compare_op=ALU.is_equal,
                fill=0.0, base=64, pattern=[[-1, 128]], channel_multiplier=1,
            )
            shd = const.tile([128, 128], F32R)
            nc.vector.tensor_copy(shd[:], shd_f[:])
            shu = const.tile([128, 128], F32R)
            nc.vector.tensor_copy(shu[:], shu_f[:])

            gam_sb = const.tile([1, H], F32)
            nc.gpsimd.dma_start(out=gam_sb[:], in_=gam_d[None, :])
            # softplus(x) = ln(1 + exp(x)); Softplus has no ACT table here
            g_e = const.tile([1, H], F32)
            nc.scalar.activation(g_e[:], gam_sb[:], AF.Exp)
            g_e1 = const.tile([1, H], F32)
            nc.vector.tensor_scalar_add(g_e1[:], g_e[:], 1.0)
            g_sp = const.tile([1, H], F32)
            nc.scalar.activation(g_sp[:], g_e1[:], AF.Ln)
            ng = const.tile([1, H], F32R)
            nc.vector.tensor_scalar_mul(ng[:], g_sp[:], -1.0)

            bmT = const.tile([128, E // 128], F32)
            nc.gpsimd.dma_start(
                out=bmT[:], in_=bm_d.rearrange("(m p) -> p m", p=128)
            )
            bo_sb = const.tile([1, E], F32R)
            nc.gpsimd.dma_start(out=bo_sb[:], in_=bo_d[None, :])

            # persistent activations (f32r: matmul operands)
            qT_sb = persist.tile([128, 4, S], F32R)    # q^T / 8, [e, s]
            kT_sb = persist.tile([128, 4, S], F32R)    # k^T
            v_aug = persist.tile([128, SQ, H, D + 1], BAND_DT)
            if shifted:
                # v rows shifted by 64: chunk sk holds v[64+128*sk + p]
                v_shift = persist.tile([128, SQ - 1, H, D + 1], BAND_DT)
            outT_sb = persist.tile([128, 4, S], F32R)  # per-head outputs^T
            WoT_sb = persist.tile([128, 4, E], F32R)
            nc.sync.dma_start(
                out=WoT_sb[:], in_=WoT_d.rearrange("(c p) e -> p c e", p=128)
            )
            ngbc = persist.tile([128, H], F32)
            colsum_sb = persist.tile([1, E], BAND_DT)

            # ---- phase 1+2: projections -----------------------------------
            with (
                tc.tile_pool(name="ph12", bufs=1) as ph12,
                tc.tile_pool(name="ps12", bufs=3, space="PSUM") as ps12,
                tc.tile_pool(name="ps12s", bufs=2, space="PSUM") as ps12s,
            ):
                combT_sb = ph12.tile([128, 8, S], F32R)
                WmT_sb = ph12.tile([128, 8, E], F32R)
                combT_r = combT_d.rearrange("(c p) s -> p c s", p=128)
                WmT_r = WmT_d.rearrange("(c p) e -> p c e", p=128)
                for c in range(8):
                    nc.sync.dma_start(
                        out=combT_sb[:, c, :], in_=combT_r[:, c, :]
                    )
                    nc.sync.dma_start(
                        out=WmT_sb[:, c, :], in_=WmT_r[:, c, :]
                    )
                WqT_sb = ph12.tile([128, 4, E], F32R)
                nc.sync.dma_start(
                    out=WqT_sb[:], in_=WqT_d.rearrange("(c p) e -> p c e", p=128)
                )
                WkT_sb = ph12.tile([128, 4, E], F32R)
                nc.sync.dma_start(
                    out=WkT_sb[:], in_=WkT_d.rearrange("(c p) e -> p c e", p=128)
                )
                WvT_sb = ph12.tile([128, 4, E], F32R)
                nc.sync.dma_start(
                    out=WvT_sb[:], in_=WvT_d.rearrange("(c p) e -> p c e", p=128)
                )

                # -g broadcast across partitions via K=1 matmul
                ps_nb = ps12s.tile([128, H], F32, tag="s")
                nc.tensor.matmul(
                    ps_nb[:], ones_row[:], ng[:], start=True, stop=True
                )
                nc.scalar.copy(ngbc[:], ps_nb[:])

                # integrated^T [e, s]
                intT_sb = ph12.tile([128, 4, S], F32R)
                for m in range(4):
                    for sh in range(2):
                        ps_i = ps12.tile([128, 512], F32, tag="mm")
                        for k in range(8):
                            nc.tensor.matmul(
                                ps_i[:],
                                WmT_sb[:, k, m * 128:(m + 1) * 128],
                                combT_sb[:, k, sh * 512:(sh + 1) * 512],
                                start=(k == 0),
                                stop=(k == 7),
                            )
                        nc.scalar.activation(
                            intT_sb[:, m, sh * 512:(sh + 1) * 512],
                            ps_i[:],
                            AF.Identity,
                            bias=bmT[:, m:m + 1],
                        )

                # q^T (scaled by 1/sqrt(D)), k^T
                for (w_sb, dst, scl) in (
                    (WqT_sb, qT_sb, 1.0 / np.sqrt(D)),
                    (WkT_sb, kT_sb, 1.0),
                ):
                    for m in range(4):
                        for sh in range(2):
                            ps_q = ps12.tile([128, 512], F32, tag="mm")
                            for k in range(4):
                                nc.tensor.matmul(
                                    ps_q[:],
                                    w_sb[:, k, m * 128:(m + 1) * 128],
                                    intT_sb[:, k, sh * 512:(sh + 1) * 512],
                                    start=(k == 0),
                                    stop=(k == 3),
                                )
                            if scl != 1.0:
                                nc.scalar.mul(
                                    dst[:, m, sh * 512:(sh + 1) * 512],
                                    ps_q[:], scl,
                                )
                            else:
                                nc.vector.tensor_copy(
                                    dst[:, m, sh * 512:(sh + 1) * 512], ps_q[:]
                                )

                # v natural [s, e] scattered into the ones-augmented layout
                for sc in range(SQ):
                    ps_v = ps12.tile([128, 512], F32, tag="mm")
                    for k in range(4):
                        nc.tensor.matmul(
                            ps_v[:],
                            intT_sb[:, k, sc * 128:(sc + 1) * 128],
                            WvT_sb[:, k, :],
                            start=(k == 0),
                            stop=(k == 3),
                        )
                    nc.scalar.copy(v_aug[:, sc, :, 0:D], ps_v[:])
                    nc.vector.tensor_copy(v_aug[:, sc, :, D], ones_fl[:, 0:H])

                # shifted v copy via two PE shift matmuls per chunk
                if shifted:
                    for sk in range(SQ - 1):
                        ps_vs = ps12.tile([128, 512], F32, tag="mm")
                        nc.tensor.matmul(
                            ps_vs[:],
                            shd[:],
                            v_aug[:, sk, :, 0:D],
                            start=True, stop=False,
                        )
                        nc.tensor.matmul(
                            ps_vs[:],
                            shu[:],
                            v_aug[:, sk + 1, :, 0:D],
                            start=False, stop=True,
                        )
                        nc.scalar.copy(v_shift[:, sk, :, 0:D], ps_vs[:])
                        nc.vector.tensor_copy(
                            v_shift[:, sk, :, D], ones_fl[:, 0:H])

                # column sums of v (all heads at once): [1, E]
                ps_cs = ps12s.tile([1, E], F32, tag="s")
                for sc in range(SQ):
                    nc.tensor.matmul(
                        ps_cs[:],
                        ones_col[:],
                        v_aug[:, sc, :, 0:D],
                        start=(sc == 0),
                        stop=(sc == SQ - 1),
                    )
                nc.scalar.copy(colsum_sb[:], ps_cs[:])

            # ---- phase 3: banded attention --------------------------------
            with (
                tc.tile_pool(name="blk", bufs=1) as blk,
                tc.tile_pool(name="work", bufs=3) as work,
                tc.tile_pool(name="attn_p", bufs=4) as attn_p,
                tc.tile_pool(name="perh", bufs=2) as perh,
                tc.tile_pool(name="ps_s", bufs=2, space="PSUM") as ps_s,
                tc.tile_pool(name="ps_t", bufs=2, space="PSUM") as ps_t,
                tc.tile_pool(name="ps_o", bufs=1, space="PSUM") as ps_o,
            ):
                # per-block persistent tiles: |i-j| strip and total tile
                # whose off-strip region stays 1.0 forever.
                dist_tiles, tot_tiles = [], []
                for qb in range(SQ):
                    sl0 = _strip(qb, W, sw)
                    dist_i = work.tile([128, sw], F32, tag="dist_i")
                    nc.gpsimd.iota(
                        dist_i[:],
                        pattern=[[-1, sw]],
                        base=qb * 128 - sl0,
                        channel_multiplier=1,
                        allow_small_or_imprecise_dtypes=True,
                    )
                    dist_a = blk.tile([128, sw], F32, tag=f"dist_a{qb}",
                                      name=f"dist_a{qb}")
                    nc.scalar.activation(dist_a[:], dist_i[:], AF.Abs)
                    dist_tiles.append(dist_a)
                    tot = blk.tile([128, sw], F32, tag=f"tot{qb}",
                                   name=f"tot{qb}")
                    tot_tiles.append(tot)

                for hp in range(4):
                    heads = (2 * hp, 2 * hp + 1)
                    mc = hp
                    bufs_bi = {}
                    bufs_be = {}
                    bufs_rT = {}
                    for h in heads:
                        if shifted:
                            bufs_bi[h] = perh.tile(
                                [128, (SQ - 1) * 256], BAND_DT,
                                tag=f"bi{h % 2}", name=f"bi{h % 2}")
                            bufs_be[h] = perh.tile(
                                [128, 4 * 128], BAND_DT,
                                tag=f"be{h % 2}", name=f"be{h % 2}")
                        else:
                            bufs_bi[h] = perh.tile(
                                [128, SQ, NSLOT * 128], BAND_DT,
                                tag=f"bi{h % 2}", name=f"bi{h % 2}")
                        bufs_rT[h] = perh.tile(
                            [1, S], BAND_DT, tag=f"rT{h % 2}",
                            name=f"rT{h % 2}")

                    for qb in range(SQ):
                        ws = _wstart(qb, W)
                        sl = _strip(qb, W, sw)
                        tot = tot_tiles[qb]
                        # paired scores on distinct PE row-groups
                        ps_pair = {}
                        for hh, h in enumerate(heads):
                            po = hh * 64
                            ps_sc = ps_s.tile([128, sw], F32, tag=f"sc{hh}",
                                              name=f"ps_sc{hh}")
                            ps_pair[h] = ps_sc
                            nc.tensor.matmul(
                                ps_sc[:],
                                qT_sb[po:po + 64, mc, qb * 128:(qb + 1) * 128],
                                kT_sb[po:po + 64, mc, sl:sl + sw],
                                start=True,
                                stop=True,
                                tile_position=(po, 0),
                            )
                        for hh, h in enumerate(heads):
                            po = hh * 64
                            # decay = exp(-g_h * dist)
                            decay = work.tile([128, sw], F32, tag="decay")
                            nc.scalar.activation(
                                decay[:], dist_tiles[qb][:], AF.Exp,
                                scale=ngbc[:, h:h + 1],
                            )
                            # sd = scores * decay; total = exp(sd) + row sums
                            sd = work.tile([128, sw], F32, tag="sd")
                            nc.vector.tensor_mul(sd[:], ps_pair[h][:], decay[:])
                            rs = work.tile([128, 1], F32, tag="rs")
                            nc.scalar.activation(
                                tot[:], sd[:], AF.Exp, accum_out=rs[:],
                            )
                            rs2 = work.tile([128, 1], F32, tag="rs2")
                            nc.vector.tensor_scalar_add(
                                rs2[:], rs[:], float(S - sw))
                            r_t = work.tile([128, 1], F32, tag="r_t")
                            nc.vector.reciprocal(r_t[:], rs2[:])

                            # attn tile: strip = tot*r, flanks = r
                            at = attn_p.tile([128, S], F32, tag="attn")
                            if hh == 0:
                                if sl > 0:
                                    nc.vector.tensor_scalar_mul(
                                        at[:, 0:sl], ones_fl[:, 0:sl], r_t[:])
                                if sl + sw < S:
                                    nc.vector.tensor_scalar_mul(
                                        at[:, sl + sw:S],
                                        ones_fl[:, 0:S - sl - sw], r_t[:])
                                nc.scalar.mul(at[:, sl:sl + sw], tot[:], r_t[:])
                            else:
                                if sl > 0:
                                    nc.scalar.mul(
                                        at[:, 0:sl], ones_fl[:, 0:sl], r_t[:])
                                if sl + sw < S:
                                    nc.vector.tensor_scalar_mul(
                                        at[:, sl + sw:S],
                                        ones_fl[:, 0:S - sl - sw], r_t[:])
                                nc.vector.tensor_scalar_mul(
                                    at[:, sl:sl + sw], tot[:], r_t[:])
                            nc.sync.dma_start(
                                out=attn_d[h, qb * 128:(qb + 1) * 128, :],
                                in_=at[:],
                            )

                            # rtb1 = (total-1)*r; window flanks -> 0
                            rtb1 = work.tile([128, W], BAND_DT, tag="rtb1")
                            lfw = sl - ws
                            rfw = ws + W - (sl + sw)
                            if lfw > 0:
                                nc.vector.tensor_copy(
                                    rtb1[:, 0:lfw], zeros_f[:, 0:lfw])
                            if rfw > 0:
                                nc.vector.tensor_copy(
                                    rtb1[:, W - rfw:W], zeros_f[:, 0:rfw])
                            nc.vector.tensor_scalar(
                                rtb1[:, lfw:lfw + sw], tot[:], 1.0,
                                r_t[:], ALU.subtract, ALU.mult,
                            )
                            # transpose the window chunks; gather
                            ps_tr = ps_t.tile([128, W], BAND_DT, tag="tr")
                            tr_chunks = []
                            for c in range(NW):
                                if not shifted:
                                    tci = ws // 128 + c
                                    slot = qb - tci + half
                                    if not (0 <= slot < NSLOT):
                                        continue
                                tr_chunks.append(c)
                                nc.tensor.transpose(
                                    ps_tr[:, c * 128:(c + 1) * 128],
                                    rtb1[:, c * 128:(c + 1) * 128],
                                    ident[:],
                                )
                            eng = (nc.scalar.copy if hh == 0
                                   else nc.vector.tensor_copy)
                            if shifted:
                                if qb == 0:
                                    dst = bufs_be[h][:, 0:256]
                                elif qb == SQ - 1:
                                    dst = bufs_be[h][:, 256:512]
                                else:
                                    dst = bufs_bi[h][:, qb * 256 - 128:
                                                     qb * 256 + 128]
                                eng(dst, ps_tr[:])
                            else:
                                c_lo = tr_chunks[0]
                                c_hi = tr_chunks[-1]
                                tci0 = ws // 128 + c_lo
                                slot0 = qb - tci0 + half
                                nch = c_hi - c_lo + 1
                                full = bufs_bi[h][:]
                                dst = bass.AP(
                                    tensor=full.tensor,
                                    offset=full.offset + tci0 * (NSLOT * 128)
                                    + slot0 * 128,
                                    ap=[full.ap[0],
                                        [(NSLOT - 1) * 128, nch],
                                        [1, 128]],
                                )
                                eng(dst, ps_tr[:, c_lo * 128:(c_hi + 1) * 128])

                    for hh, h in enumerate(heads):
                        po = hh * 64
                        rT_all = bufs_rT[h]
                        # out^T_h[d,q] (+ row 64 = band sums) over all q
                        ps_ov = ps_o.tile([65, S], F32, tag="ov")
                        for j0 in range(0, S, 512):
                            nc.tensor.matmul(
                                ps_ov[:, j0:j0 + 512],
                                zeros_a[:],
                                zrow[:],
                                start=True,
                                stop=False,
                            )
                        if shifted:
                            for sk in range(SQ - 1):
                                # qb slots present in the interior buffer:
                                # qb=0 and qb=SQ-1 live in the edge buffer
                                c0 = sk * 128 if sk > 0 else 128
                                c1 = sk * 128 + (256 if sk < SQ - 2 else 128)
                                p = c0
                                while p < c1:
                                    pe = min(c1, (p // 512 + 1) * 512)
                                    nc.tensor.matmul(
                                        ps_ov[:, p:pe],
                                        v_shift[:, sk, h, :],
                                        bufs_bi[h][:, sk * 128 + p:
                                                   sk * 128 + pe],
                                        start=False,
                                        stop=False,
                                    )
                                    p = pe
                            for ei, (vc, q0) in enumerate(
                                    ((0, 0), (1, 0), (SQ - 2, SQ - 1),
                                     (SQ - 1, SQ - 1))):
                                nc.tensor.matmul(
                                    ps_ov[:, q0 * 128:(q0 + 1) * 128],
                                    v_aug[:, vc, h, :],
                                    bufs_be[h][:, ei * 128:(ei + 1) * 128],
                                    start=False,
                                    stop=False,
                                )
                        else:
                            for tci in range(SQ):
                                qlo = max(tci - half, 0)
                                qhi = min(tci + half, SQ - 1)
                                slo = qlo - tci + half
                                c0, c1 = qlo * 128, (qhi + 1) * 128
                                p = c0
                                while p < c1:
                                    pe = min(c1, (p // 512 + 1) * 512)
                                    nc.tensor.matmul(
                                        ps_ov[:, p:pe],
                                        v_aug[:, tci, h, :],
                                        bufs_bi[h][:, tci,
                                                   (slo * 128 + p - c0):
                                                   (slo * 128 + pe - c0)],
                                        start=False,
                                        stop=False,
                                    )
                                    p = pe
                        # r^T row = (1 - band_sum) / S, from psum row 64
                        brow = work.tile([1, S], F32, tag="brow")
                        nc.vector.tensor_copy(brow[:], ps_ov[64:65, :])
                        nc.vector.tensor_scalar(
                            rT_all[:], brow[:], 1.0, -1.0 / S,
                            ALU.subtract, ALU.mult,
                        )
                        # off-band term: colsum x r^T accumulated on top
                        for j0 in range(0, S, 512):
                            nc.tensor.matmul(
                                ps_ov[0:64, j0:j0 + 512],
                                colsum_sb[0:1, h * 64:(h + 1) * 64],
                                rT_all[0:1, j0:j0 + 512],
                                start=False,
                                stop=(j0 == 512),
                            )
                        nc.scalar.copy(
                            outT_sb[po:po + 64, hp, :], ps_ov[0:64, :]
                        )

            # ---- phase 4: output projection -------------------------------
            with (
                tc.tile_pool(name="ph4", bufs=3) as ph4,
                tc.tile_pool(name="ps4", bufs=2, space="PSUM") as ps4,
            ):
                for qb in range(SQ):
                    ps_f = ps4.tile([128, E], F32)
                    for ec in range(4):
                        nc.tensor.matmul(
                            ps_f[:],
                            outT_sb[:, ec, qb * 128:(qb + 1) * 128],
                            WoT_sb[:, ec, :],
                            start=(ec == 0),
                            stop=False,
                        )
                    nc.tensor.matmul(
                        ps_f[:], ones_row[:], bo_sb[:], start=False, stop=True
                    )
                    o_sb = ph4.tile([128, E], F32, tag="o")
                    nc.scalar.copy(o_sb[:], ps_f[:])
                    nc.sync.dma_start(
                        out=out_d[qb * 128:(qb + 1) * 128, :], in_=o_sb[:]
                    )

    nc.compile()
    return nc


def kernel(**inputs):
    x = np.asarray(inputs["unified_embed"], dtype=np.float32)
    mem = np.asarray(inputs["memory_state"], dtype=np.float32)
    Wq = np.asarray(inputs["Wq"], dtype=np.float32)
    Wk = np.asarray(inputs["Wk"], dtype=np.float32)
    Wv = np.asarray(inputs["Wv"], dtype=np.float32)
    Wm = np.asarray(inputs["Wm"], dtype=np.float32)
    bm = np.asarray(inputs["bm"], dtype=np.float32)
    Wo = np.asarray(inputs["Wo"], dtype=np.float32)
    bo = np.asarray(inputs["bo"], dtype=np.float32)
    gammas = np.asarray(inputs["gammas"], dtype=np.float32)

    g = np.logaddexp(0.0, gammas.astype(np.float64))  # softplus, host-side
    g_min = float(g.min())
    W = _pick_window(g_min)
    dcut = int(np.ceil(22.18 / g_min))
    sw = min(W, ((128 + 2 * dcut + 7) // 8) * 8)

    key = (W, sw)
    if key not in _CACHE:
        _CACHE[key] = _build(W, sw)
    nc = _CACHE[key]

    WmT = np.ascontiguousarray(Wm.T)
    WqT = np.ascontiguousarray(Wq.T)
    WkT = np.ascontiguousarray(Wk.T)
    WvT = np.ascontiguousarray(Wv.T)
    WoT = np.ascontiguousarray(Wo.T)

    in_maps = []
    for b in range(NCORES):
        combT = np.ascontiguousarray(
            np.concatenate([x[b], mem[b]], axis=1).T
        )
        in_maps.append({
            "combT": combT,
            "WmT": WmT, "WqT": WqT, "WkT": WkT, "WvT": WvT, "WoT": WoT,
            "bm": bm, "bo": bo, "gam": gammas,
        })

    res = run_bass_kernel_spmd(
        nc, in_maps, core_ids=list(range(NCORES)),
        trace=bool(int(os.environ.get("KERNEL_TRACE", "0"))),
    )
    LAST_PERF["exec_time_ns"] = res.exec_time_ns
    LAST_PERF["mean_exec_time_ns"] = res.mean_exec_time_ns
    LAST_PERF["trace"] = res.instructions_and_trace

    out = np.stack([res.results[b]["out"] for b in range(NCORES)])
    attn = np.stack([res.results[b]["attn"] for b in range(NCORES)])
    return out, attn


# revision 25
# speedup vs baseline: 1.0897x; 1.0897x over previous
"""Trainium2 Bass kernel for nn_MemoryAwareAKTAttention.

Math (per batch b):
    integrated = concat([x, mem], -1) @ Wm.T + bm          [S, E]
    q, k, v    = heads(integrated @ W{q,k,v}.T)            [H, S, D]
    scores     = q @ k.T / sqrt(D)                         [H, S, S]
    decay      = exp(-softplus(gamma_h) * |i-j|)           [H, S, S]
    total      = clip(exp(scores * decay), 1e-5, 1e5)
    attn       = total / (sum_j total + 1e-9)
    out        = (attn @ v) @ Wo.T + bo                    [S, E]

Structure exploited: decay underflows to exactly 0.0 a few dozen columns off
the diagonal, so total == 1.0f bit-exactly outside a narrow band (exp(x)
rounds to 1.0f for |x| < 2^-25).  Per 128-row query block only a ~168-wide
strip is computed (scores matmul + exp); a persistent [128, S] total tile
holds 1.0 elsewhere.  attn is a single row-scale of that tile.  For
out = attn @ v, the off-band part folds into a per-head column-sum of v
(rank-1 colsum x r^T), and the row sums fall out of the banded matmul via a
ones-column appended to v: sum_j (total-1)*r = 1 - S*r.  The banded matmul
contracts 256-wide windows, half-chunk shifted, against a row-shifted copy
of v built once with two PE shift-matmuls.

Matmuls run in float32r (fp32 is 4 cycles/row, f32r is 2); the exp and
normalization chain stays fp32.  attn comes out ~6e-4, out ~4e-4 max rel err.

Sharding: data-parallel over batch, one batch element per NeuronCore (B=8).
"""

import os
import numpy as np

import concourse.bass as bass
import concourse.mybir as mybir
import concourse.tile as tile
from concourse import bacc
from concourse.bass_utils import run_bass_kernel_spmd
from concourse.masks import make_identity

F32 = mybir.dt.float32
F32R = mybir.dt.float32r
BF16 = mybir.dt.bfloat16
BAND_DT = F32R     # dtype of the (total-1)*r band / v / colsum path

AF = mybir.ActivationFunctionType
ALU = mybir.AluOpType

B, S, E = 8, 1024, 512
H = 8
D = E // H          # 64
TWO_E = 2 * E
NCORES = 8
SQ = S // 128       # query blocks per core

LAST_PERF = {}      # filled by kernel(): exec_time_ns etc.
_CACHE = {}         # (W, sw) -> compiled Bacc module


def _wstart(qb: int, W: int) -> int:
    """Window start for query block qb, clipped to [0, S-W].

    W=256 windows are half-chunk shifted: [qb*128-64, qb*128+192).
    Odd-chunk windows are chunk-aligned and centered.
    """
    n = W // 128
    if n == 2:
        return min(max(qb * 128 - 64, 0), S - W)
    return min(max((qb - (n - 1) // 2) * 128, 0), S - W)


def _strip(qb: int, W: int, sw: int) -> int:
    """Start of the compute strip (columns where decay can exceed 2^-31)."""
    dcut = (sw - 128) // 2
    lo = min(max(qb * 128 - dcut, 0), S - sw)
    ws = _wstart(qb, W)
    return min(max(lo, ws), ws + W - sw)


def _pick_window(g_min: float) -> int:
    """Smallest window such that off-window total == 1.0f exactly.

    need exp(-g*margin) * SMAX < 2^-25 with SMAX = 64; ln(64 * 2^26) ~ 22.18.
    """
    for n, margin in ((2, 65), (3, 129), (5, 257), (7, 385)):
        if g_min * margin >= 22.18:
            return n * 128
    return S  # dense fallback: every column computed exactly


def _build(W: int, sw: int) -> bacc.Bacc:
    nc = bacc.Bacc(None)
    NW = W // 128
    shifted = (NW == 2)
    half = (NW - 1) // 2
    NSLOT = 2 * half + 1

    combT_d = nc.dram_tensor("combT", [TWO_E, S], F32R, kind="ExternalInput")
    WmT_d = nc.dram_tensor("WmT", [TWO_E, E], F32R, kind="ExternalInput")
    WqT_d = nc.dram_tensor("WqT", [E, E], F32R, kind="ExternalInput")
    WkT_d = nc.dram_tensor("WkT", [E, E], F32R, kind="ExternalInput")
    WvT_d = nc.dram_tensor("WvT", [E, E], F32R, kind="ExternalInput")
    WoT_d = nc.dram_tensor("WoT", [E, E], F32R, kind="ExternalInput")
    bm_d = nc.dram_tensor("bm", [E], F32, kind="ExternalInput")
    bo_d = nc.dram_tensor("bo", [E], F32R, kind="ExternalInput")
    gam_d = nc.dram_tensor("gam", [H], F32, kind="ExternalInput")
    out_d = nc.dram_tensor("out", [S, E], F32, kind="ExternalOutput")
    attn_d = nc.dram_tensor("attn", [H, S, S], F32, kind="ExternalOutput")

    with tile.TileContext(nc) as tc:
        with (
            tc.tile_pool(name="const", bufs=1) as const,
            tc.tile_pool(name="persist", bufs=1) as persist,
        ):
            # ---- constants ------------------------------------------------
            ident_f = const.tile([128, 128], F32)
            make_identity(nc, ident_f[:])
            ident = const.tile([128, 128], F32R)
            nc.vector.tensor_copy(ident[:], ident_f[:])
            ones_fl = const.tile([128, S], F32)
            nc.vector.memset(ones_fl[:], 1.0)
            ones_row = const.tile([1, 128], F32R)
            nc.vector.tensor_copy(ones_row[:], ones_fl[0:1, 0:128])
            zeros_f = const.tile([128, 128], F32)
            nc.vector.memset(zeros_f[:], 0.0)
            zeros_a = const.tile([1, 65], BF16)
            nc.vector.tensor_copy(zeros_a[:], zeros_f[0:1, 0:65])
            zrow = const.tile([1, 512], BF16)
            nc.vector.tensor_copy(zrow[:], ones_fl[0:1, 0:512])
            ones_col = const.tile([128, 1], BAND_DT)
            nc.vector.tensor_copy(ones_col[:], ones_fl[:, 0:1])
            ones512b = const.tile([1, 512], BAND_DT)
            nc.vector.tensor_copy(ones512b[:], ones_fl[0:1, 0:512])
            # shift matrices: shd[p, m] = 1{p == m+64}, shu[p, m] = 1{p == m-64}
            shd_f = const.tile([128, 128], F32)
            nc.gpsimd.memset(shd_f[:], 1.0)
            nc.gpsimd.affine_select(
                out=shd_f[:], in_=shd_f[:], compare_op=ALU.is_equal,
                fill=0.0, base=-64, pattern=[[-1, 128]], channel_multiplier=1,
            )
            shu_f = const.tile([128, 128], F32)
            nc.gpsimd.memset(shu_f[:], 1.0)
            nc.gpsimd.affine_select(
                out=shu_f[:], in_=shu_f[:], compare_op=ALU.is_equal,
                fill=0.0, base=64, pattern=[[-1, 128]], channel_multiplier=1,
            )
            shd = const.tile([128, 128], F32R)
            nc.vector.tensor_copy(shd[:], shd_f[:])
            shu = const.tile([128, 128], F32R)
            nc.vector.tensor_copy(shu[:], shu_f[:])

            gam_sb = const.tile([1, H], F32)
            nc.gpsimd.dma_start(out=gam_sb[:], in_=gam_d[None, :])
            # softplus(x) = ln(1 + exp(x)); Softplus has no ACT table here
            g_e = const.tile([1, H], F32)
            nc.scalar.activation(g_e[:], gam_sb[:], AF.Exp)
            g_e1 = const.tile([1, H], F32)
            nc.vector.tensor_scalar_add(g_e1[:], g_e[:], 1.0)
            g_sp = const.tile([1, H], F32)
            nc.scalar.activation(g_sp[:], g_e1[:], AF.Ln)
            ng = const.tile([1, H], F32R)
            nc.vector.tensor_scalar_mul(ng[:], g_sp[:], -1.0)

            bmT = const.tile([128, E // 128], F32)
            nc.gpsimd.dma_start(
                out=bmT[:], in_=bm_d.rearrange("(m p) -> p m", p=128)
            )
            bo_sb = const.tile([1, E], F32R)
            nc.gpsimd.dma_start(out=bo_sb[:], in_=bo_d[None, :])

            # persistent activations (f32r: matmul operands)
            qT_sb = persist.tile([128, 4, S], F32R)    # q^T / 8, [e, s]
            kT_sb = persist.tile([128, 4, S], F32R)    # k^T
            v_aug = persist.tile([128, SQ, H, D + 1], BAND_DT)
            if shifted:
                # v rows shifted by 64: chunk sk holds v[64+128*sk + p]
                v_shift = persist.tile([128, SQ - 1, H, D + 1], BAND_DT)
            outT_sb = persist.tile([128, 4, S], F32R)  # per-head outputs^T
            WoT_sb = persist.tile([128, 4, E], F32R)
            nc.sync.dma_start(
                out=WoT_sb[:], in_=WoT_d.rearrange("(c p) e -> p c e", p=128)
            )
            ngbc = persist.tile([128, H], F32)
            colsum_sb = persist.tile([1, E], BAND_DT)   # colsum / S
            colsum_n = persist.tile([1, E], BAND_DT)    # -colsum / S

            # ---- phase 1+2: projections -----------------------------------
            with (
                tc.tile_pool(name="ph12", bufs=1) as ph12,
                tc.tile_pool(name="ps12", bufs=3, space="PSUM") as ps12,
                tc.tile_pool(name="ps12s", bufs=2, space="PSUM") as ps12s,
            ):
                combT_sb = ph12.tile([128, 8, S], F32R)
                WmT_sb = ph12.tile([128, 8, E], F32R)
                combT_r = combT_d.rearrange("(c p) s -> p c s", p=128)
                WmT_r = WmT_d.rearrange("(c p) e -> p c e", p=128)
                for c in range(8):
                    nc.sync.dma_start(
                        out=combT_sb[:, c, :], in_=combT_r[:, c, :]
                    )
                    nc.sync.dma_start(
                        out=WmT_sb[:, c, :], in_=WmT_r[:, c, :]
                    )
                WqT_sb = ph12.tile([128, 4, E], F32R)
                nc.sync.dma_start(
                    out=WqT_sb[:], in_=WqT_d.rearrange("(c p) e -> p c e", p=128)
                )
                WkT_sb = ph12.tile([128, 4, E], F32R)
                nc.sync.dma_start(
                    out=WkT_sb[:], in_=WkT_d.rearrange("(c p) e -> p c e", p=128)
                )
                WvT_sb = ph12.tile([128, 4, E], F32R)
                nc.sync.dma_start(
                    out=WvT_sb[:], in_=WvT_d.rearrange("(c p) e -> p c e", p=128)
                )

                # -g broadcast across partitions via K=1 matmul
                ps_nb = ps12s.tile([128, H], F32, tag="s")
                nc.tensor.matmul(
                    ps_nb[:], ones_row[:], ng[:], start=True, stop=True
                )
                nc.scalar.copy(ngbc[:], ps_nb[:])

                # integrated^T [e, s]
                intT_sb = ph12.tile([128, 4, S], F32R)
                for m in range(4):
                    for sh in range(2):
                        ps_i = ps12.tile([128, 512], F32, tag="mm")
                        for k in range(8):
                            nc.tensor.matmul(
                                ps_i[:],
                                WmT_sb[:, k, m * 128:(m + 1) * 128],
                                combT_sb[:, k, sh * 512:(sh + 1) * 512],
                                start=(k == 0),
                                stop=(k == 7),
                            )
                        nc.scalar.activation(
                            intT_sb[:, m, sh * 512:(sh + 1) * 512],
                            ps_i[:],
                            AF.Identity,
                            bias=bmT[:, m:m + 1],
                        )

                # q^T (scaled by 1/sqrt(D)), k^T
                for (w_sb, dst, scl) in (
                    (WqT_sb, qT_sb, 1.0 / np.sqrt(D)),
                    (WkT_sb, kT_sb, 1.0),
                ):
                    for m in range(4):
                        for sh in range(2):
                            ps_q = ps12.tile([128, 512], F32, tag="mm")
                            for k in range(4):
                                nc.tensor.matmul(
                                    ps_q[:],
                                    w_sb[:, k, m * 128:(m + 1) * 128],
                                    intT_sb[:, k, sh * 512:(sh + 1) * 512],
                                    start=(k == 0),
                                    stop=(k == 3),
                                )
                            if scl != 1.0:
                                nc.scalar.mul(
                                    dst[:, m, sh * 512:(sh + 1) * 512],
                                    ps_q[:], scl,
                                )
                            else:
                                nc.vector.tensor_copy(
                                    dst[:, m, sh * 512:(sh + 1) * 512], ps_q[:]
                                )

                # v natural [s, e] scattered into the ones-augmented layout
                for sc in range(SQ):
                    ps_v = ps12.tile([128, 512], F32, tag="mm")
                    for k in range(4):
                        nc.tensor.matmul(
                            ps_v[:],
                            intT_sb[:, k, sc * 128:(sc + 1) * 128],
                            WvT_sb[:, k, :],
                            start=(k == 0),
                            stop=(k == 3),
                        )
                    nc.scalar.copy(v_aug[:, sc, :, 0:D], ps_v[:])
                    nc.vector.tensor_copy(v_aug[:, sc, :, D], ones_fl[:, 0:H])

                # shifted v copy via two PE shift matmuls per chunk
                if shifted:
                    for sk in range(SQ - 1):
                        ps_vs = ps12.tile([128, 512], F32, tag="mm")
                        nc.tensor.matmul(
                            ps_vs[:],
                            shd[:],
                            v_aug[:, sk, :, 0:D],
                            start=True, stop=False,
                        )
                        nc.tensor.matmul(
                            ps_vs[:],
                            shu[:],
                            v_aug[:, sk + 1, :, 0:D],
                            start=False, stop=True,
                        )
                        nc.scalar.copy(v_shift[:, sk, :, 0:D], ps_vs[:])
                        nc.vector.tensor_copy(
                            v_shift[:, sk, :, D], ones_fl[:, 0:H])

                # column sums of v (all heads at once): [1, E]
                ps_cs = ps12s.tile([1, E], F32, tag="s")
                for sc in range(SQ):
                    nc.tensor.matmul(
                        ps_cs[:],
                        ones_col[:],
                        v_aug[:, sc, :, 0:D],
                        start=(sc == 0),
                        stop=(sc == SQ - 1),
                    )
                nc.vector.tensor_scalar_mul(colsum_sb[:], ps_cs[:], 1.0 / S)
                nc.vector.tensor_scalar_mul(colsum_n[:], ps_cs[:], -1.0 / S)

            # ---- phase 3: banded attention --------------------------------
            with (
                tc.tile_pool(name="blk", bufs=1) as blk,
                tc.tile_pool(name="work", bufs=3) as work,
                tc.tile_pool(name="attn_p", bufs=4) as attn_p,
                tc.tile_pool(name="perh", bufs=2) as perh,
                tc.tile_pool(name="ps_s", bufs=1, space="PSUM") as ps_s,
                tc.tile_pool(name="ps_t", bufs=2, space="PSUM") as ps_t,
                tc.tile_pool(name="ps_o", bufs=2, space="PSUM") as ps_o,
            ):
                # per-block persistent tiles: |i-j| strip and total tile
                # whose off-strip region stays 1.0 forever.
                dist_tiles, tot_tiles = [], []
                for qb in range(SQ):
                    sl0 = _strip(qb, W, sw)
                    dist_i = work.tile([128, sw], F32, tag="dist_i")
                    nc.gpsimd.iota(
                        dist_i[:],
                        pattern=[[-1, sw]],
                        base=qb * 128 - sl0,
                        channel_multiplier=1,
                        allow_small_or_imprecise_dtypes=True,
                    )
                    dist_a = blk.tile([128, sw], F32, tag=f"dist_a{qb}",
                                      name=f"dist_a{qb}")
                    nc.scalar.activation(dist_a[:], dist_i[:], AF.Abs)
                    dist_tiles.append(dist_a)
                    tot = blk.tile([128, sw], F32, tag=f"tot{qb}",
                                   name=f"tot{qb}")
                    tot_tiles.append(tot)

                for hp in range(4):
                    heads = (2 * hp, 2 * hp + 1)
                    mc = hp
                    bufs_bi = {}
                    bufs_be = {}
                    for h in heads:
                        if shifted:
                            bufs_bi[h] = perh.tile(
                                [128, (SQ - 1) * 256], BAND_DT,
                                tag=f"bi{h % 2}", name=f"bi{h % 2}")
                            bufs_be[h] = perh.tile(
                                [128, 4 * 128], BAND_DT,
                                tag=f"be{h % 2}", name=f"be{h % 2}")
                        else:
                            bufs_bi[h] = perh.tile(
                                [128, SQ, NSLOT * 128], BAND_DT,
                                tag=f"bi{h % 2}", name=f"bi{h % 2}")


                    for qb in range(SQ):
                        ws = _wstart(qb, W)
                        sl = _strip(qb, W, sw)
                        tot = tot_tiles[qb]
                        # paired scores on distinct PE row-groups
                        ps_pair = {}
                        for hh, h in enumerate(heads):
                            po = hh * 64
                            ps_sc = ps_s.tile([128, sw], F32, tag=f"sc{hh}",
                                              name=f"ps_sc{hh}")
                            ps_pair[h] = ps_sc
                            nc.tensor.matmul(
                                ps_sc[:],
                                qT_sb[po:po + 64, mc, qb * 128:(qb + 1) * 128],
                                kT_sb[po:po + 64, mc, sl:sl + sw],
                                start=True,
                                stop=True,
                                tile_position=(po, 0),
                            )
                        for hh, h in enumerate(heads):
                            po = hh * 64
                            # decay = exp(-g_h * dist)
                            decay = work.tile([128, sw], F32, tag="decay")
                            nc.scalar.activation(
                                decay[:], dist_tiles[qb][:], AF.Exp,
                                scale=ngbc[:, h:h + 1],
                            )
                            # sd = scores * decay; total = exp(sd) + row sums
                            sd = work.tile([128, sw], F32, tag="sd")
                            nc.vector.tensor_mul(sd[:], ps_pair[h][:], decay[:])
                            rs = work.tile([128, 1], F32, tag="rs")
                            nc.scalar.activation(
                                tot[:], sd[:], AF.Exp, accum_out=rs[:],
                            )
                            rs2 = work.tile([128, 1], F32, tag="rs2")
                            nc.vector.tensor_scalar_add(
                                rs2[:], rs[:], float(S - sw))
                            r_t = work.tile([128, 1], F32, tag="r_t")
                            nc.vector.reciprocal(r_t[:], rs2[:])

                            # attn tile: strip = tot*r, flanks = r
                            at = attn_p.tile([128, S], F32, tag="attn")
                            if hh == 0:
                                if sl > 0:
                                    nc.vector.tensor_scalar_mul(
                                        at[:, 0:sl], ones_fl[:, 0:sl], r_t[:])
                                if sl + sw < S:
                                    nc.vector.tensor_scalar_mul(
                                        at[:, sl + sw:S],
                                        ones_fl[:, 0:S - sl - sw], r_t[:])
                                nc.scalar.mul(at[:, sl:sl + sw], tot[:], r_t[:])
                            else:
                                if sl > 0:
                                    nc.scalar.mul(
                                        at[:, 0:sl], ones_fl[:, 0:sl], r_t[:])
                                if sl + sw < S:
                                    nc.vector.tensor_scalar_mul(
                                        at[:, sl + sw:S],
                                        ones_fl[:, 0:S - sl - sw], r_t[:])
                                nc.vector.tensor_scalar_mul(
                                    at[:, sl:sl + sw], tot[:], r_t[:])
                            nc.sync.dma_start(
                                out=attn_d[h, qb * 128:(qb + 1) * 128, :],
                                in_=at[:],
                            )

                            # rtb1 = (total-1)*r; window flanks -> 0
                            rtb1 = work.tile([128, W], BAND_DT, tag="rtb1")
                            lfw = sl - ws
                            rfw = ws + W - (sl + sw)
                            if lfw > 0:
                                nc.vector.tensor_copy(
                                    rtb1[:, 0:lfw], zeros_f[:, 0:lfw])
                            if rfw > 0:
                                nc.vector.tensor_copy(
                                    rtb1[:, W - rfw:W], zeros_f[:, 0:rfw])
                            nc.vector.tensor_scalar(
                                rtb1[:, lfw:lfw + sw], tot[:], 1.0,
                                r_t[:], ALU.subtract, ALU.mult,
                            )
                            # transpose the window chunks; gather
                            ps_tr = ps_t.tile([128, W], BAND_DT, tag="tr")
                            tr_chunks = []
                            for c in range(NW):
                                if not shifted:
                                    tci = ws // 128 + c
                                    slot = qb - tci + half
                                    if not (0 <= slot < NSLOT):
                                        continue
                                tr_chunks.append(c)
                                nc.tensor.transpose(
                                    ps_tr[:, c * 128:(c + 1) * 128],
                                    rtb1[:, c * 128:(c + 1) * 128],
                                    ident[:],
                                )
                            eng = (nc.scalar.copy if hh == 0
                                   else nc.vector.tensor_copy)
                            if shifted:
                                if qb == 0:
                                    dst = bufs_be[h][:, 0:256]
                                elif qb == SQ - 1:
                                    dst = bufs_be[h][:, 256:512]
                                else:
                                    dst = bufs_bi[h][:, qb * 256 - 128:
                                                     qb * 256 + 128]
                                eng(dst, ps_tr[:])
                            else:
                                c_lo = tr_chunks[0]
                                c_hi = tr_chunks[-1]
                                tci0 = ws // 128 + c_lo
                                slot0 = qb - tci0 + half
                                nch = c_hi - c_lo + 1
                                full = bufs_bi[h][:]
                                dst = bass.AP(
                                    tensor=full.tensor,
                                    offset=full.offset + tci0 * (NSLOT * 128)
                                    + slot0 * 128,
                                    ap=[full.ap[0],
                                        [(NSLOT - 1) * 128, nch],
                                        [1, 128]],
                                )
                                eng(dst, ps_tr[:, c_lo * 128:(c_hi + 1) * 128])

                    for hh, h in enumerate(heads):
                        po = hh * 64
                        # out^T_h[d,q] (+ row 64 = band sums) over all q
                        ps_ov = ps_o.tile([65, S], F32, tag="ov")
                        for j0 in range(0, S, 512):
                            nc.tensor.matmul(
                                ps_ov[:, j0:j0 + 512],
                                zeros_a[:],
                                zrow[:],
                                start=True,
                                stop=False,
                            )
                        if shifted:
                            for sk in range(SQ - 1):
                                # qb slots present in the interior buffer:
                                # qb=0 and qb=SQ-1 live in the edge buffer
                                c0 = sk * 128 if sk > 0 else 128
                                c1 = sk * 128 + (256 if sk < SQ - 2 else 128)
                                p = c0
                                while p < c1:
                                    pe = min(c1, (p // 512 + 1) * 512)
                                    nc.tensor.matmul(
                                        ps_ov[:, p:pe],
                                        v_shift[:, sk, h, :],
                                        bufs_bi[h][:, sk * 128 + p:
                                                   sk * 128 + pe],
                                        start=False,
                                        stop=False,
                                    )
                                    p = pe
                            for ei, (vc, q0) in enumerate(
                                    ((0, 0), (1, 0), (SQ - 2, SQ - 1),
                                     (SQ - 1, SQ - 1))):
                                nc.tensor.matmul(
                                    ps_ov[:, q0 * 128:(q0 + 1) * 128],
                                    v_aug[:, vc, h, :],
                                    bufs_be[h][:, ei * 128:(ei + 1) * 128],
                                    start=False,
                                    stop=False,
                                )
                        else:
                            for tci in range(SQ):
                                qlo = max(tci - half, 0)
                                qhi = min(tci + half, SQ - 1)
                                slo = qlo - tci + half
                                c0, c1 = qlo * 128, (qhi + 1) * 128
                                p = c0
                                while p < c1:
                                    pe = min(c1, (p // 512 + 1) * 512)
                                    nc.tensor.matmul(
                                        ps_ov[:, p:pe],
                                        v_aug[:, tci, h, :],
                                        bufs_bi[h][:, tci,
                                                   (slo * 128 + p - c0):
                                                   (slo * 128 + pe - c0)],
                                        start=False,
                                        stop=False,
                                    )
                                    p = pe
                        # off-band term: colsum x r^T with
                        # r^T = (1 - band_sum)/S expanded into two rank-1
                        # matmuls (avoids a serial [1,S] rescale)
                        brow = work.tile([1, S], BAND_DT, tag="brow")
                        eng2 = (nc.scalar.copy if hh == 0
                                else nc.vector.tensor_copy)
                        eng2(brow[:], ps_ov[64:65, :])
                        for j0 in range(0, S, 512):
                            nc.tensor.matmul(
                                ps_ov[0:64, j0:j0 + 512],
                                colsum_sb[0:1, h * 64:(h + 1) * 64],
                                ones512b[:],
                                start=False,
                                stop=False,
                            )
                            nc.tensor.matmul(
                                ps_ov[0:64, j0:j0 + 512],
                                colsum_n[0:1, h * 64:(h + 1) * 64],
                                brow[0:1, j0:j0 + 512],
                                start=False,
                                stop=(j0 == 512),
                            )
                        eng3 = (nc.vector.tensor_copy if hh == 0
                                else nc.scalar.copy)
                        eng3(
                            outT_sb[po:po + 64, hp, :], ps_ov[0:64, :]
                        )

            # ---- phase 4: output projection -------------------------------
            with (
                tc.tile_pool(name="ph4", bufs=3) as ph4,
                tc.tile_pool(name="ps4", bufs=2, space="PSUM") as ps4,
            ):
                for qb in range(SQ):
                    ps_f = ps4.tile([128, E], F32)
                    for ec in range(4):
                        nc.tensor.matmul(
                            ps_f[:],
                            outT_sb[:, ec, qb * 128:(qb + 1) * 128],
                            WoT_sb[:, ec, :],
                            start=(ec == 0),
                            stop=False,
                        )
                    nc.tensor.matmul(
                        ps_f[:], ones_row[:], bo_sb[:], start=False, stop=True
                    )
                    o_sb = ph4.tile([128, E], F32, tag="o")
                    nc.scalar.copy(o_sb[:], ps_f[:])
                    nc.sync.dma_start(
                        out=out_d[qb * 128:(qb + 1) * 128, :], in_=o_sb[:]
                    )

    nc.compile()
    return nc


def kernel(**inputs):
    x = np.asarray(inputs["unified_embed"], dtype=np.float32)
    mem = np.asarray(inputs["memory_state"], dtype=np.float32)
    Wq = np.asarray(inputs["Wq"], dtype=np.float32)
    Wk = np.asarray(inputs["Wk"], dtype=np.float32)
    Wv = np.asarray(inputs["Wv"], dtype=np.float32)
    Wm = np.asarray(inputs["Wm"], dtype=np.float32)
    bm = np.asarray(inputs["bm"], dtype=np.float32)
    Wo = np.asarray(inputs["Wo"], dtype=np.float32)
    bo = np.asarray(inputs["bo"], dtype=np.float32)
    gammas = np.asarray(inputs["gammas"], dtype=np.float32)

    g = np.logaddexp(0.0, gammas.astype(np.float64))  # softplus, host-side
    g_min = float(g.min())
    W = _pick_window(g_min)
    dcut = int(np.ceil(22.18 / g_min))
    sw = min(W, ((128 + 2 * dcut + 7) // 8) * 8)

    key = (W, sw)
    if key not in _CACHE:
        _CACHE[key] = _build(W, sw)
    nc = _CACHE[key]

    WmT = np.ascontiguousarray(Wm.T)
    WqT = np.ascontiguousarray(Wq.T)
    WkT = np.ascontiguousarray(Wk.T)
    WvT = np.ascontiguousarray(Wv.T)
    WoT = np.ascontiguousarray(Wo.T)

    in_maps = []
    for b in range(NCORES):
        combT = np.ascontiguousarray(
            np.concatenate([x[b], mem[b]], axis=1).T
        )
        in_maps.append({
            "combT": combT,
            "WmT": WmT, "WqT": WqT, "WkT": WkT, "WvT": WvT, "WoT": WoT,
            "bm": bm, "bo": bo, "gam": gammas,
        })

    res = run_bass_kernel_spmd(
        nc, in_maps, core_ids=list(range(NCORES)),
        trace=bool(int(os.environ.get("KERNEL_TRACE", "0"))),
    )
    LAST_PERF["exec_time_ns"] = res.exec_time_ns
    LAST_PERF["mean_exec_time_ns"] = res.mean_exec_time_ns
    LAST_PERF["trace"] = res.instructions_and_trace

    out = np.stack([res.results[b]["out"] for b in range(NCORES)])
    attn = np.stack([res.results[b]["attn"] for b in range(NCORES)])
    return out, attn
